# revision 12
# baseline (speedup 1.0000x reference)
"""Trainium2 Bass kernel for nn_GATLayer (GATv2 message passing + LayerNorm).

Strategy (row/edge hybrid parallel, 8 cores):
  - Pad the 20000 destination rows to 20480 = 8 cores x 20 windows x 128 rows.
    Core k owns rows [2560k, 2560(k+1)) and ALL edges pointing at them.
  - Host prep: sort edges by dst, group them into 128-row windows, pad each
    window's edge list to a common W_E (multiple of 512), gather token_x[src]
    and edge_attr per edge, build onehot connectivity matrices (row-major and
    col-major), and ship per-core transposed bf16 streams.
  - Device, per window: e = x_src@w_l + ea@w_e + ohr^T-expand(hd), lrelu
    (Prelu, alpha=0.2), score via block-diag att matmuls into one PSUM bank,
    w = exp(score) once per window (no max-subtraction needed: out =
    sum(hs*w)/sum(w) is softmax-shift-invariant and scores are O(5)),
    msg = hs_col * w, segment-reduce via onehot_col matmuls accumulated in
    PSUM, then divide, residual add, LayerNorm -> 2560 output rows per core.
  - No collectives: each core owns its rows end-to-end; host concatenates.
"""
import numpy as np
import ml_dtypes

bf16 = ml_dtypes.bfloat16

# Problem constants (hardcoded per harness contract)
N_ROWS, N_TOK, E, D, ED, H, C = 20000, 200000, 1_000_000, 128, 64, 4, 32
NEG_SLOPE, LN_EPS = 0.2, 1e-5
NCORES = 8
WIN = 128                       # rows per window (PSUM partition dim)
ROWS_PAD = 20480                # 160 windows
ROWS_PER_CORE = ROWS_PAD // NCORES   # 2560
NWIN = ROWS_PER_CORE // WIN          # 20
NW_TOT = ROWS_PAD // WIN             # 160
TILE = 512                      # edges per PE moving-operand tile

_prog_cache = {}
last_run = None                 # BassKernelResults of the most recent run


def _build_program(W_E):
    import concourse.tile as tile
    from concourse import bacc, mybir

    F32, BF = mybir.dt.float32, mybir.dt.bfloat16
    AF = mybir.ActivationFunctionType
    ALU = mybir.AluOpType

    NBLK = W_E // 128
    NT = W_E // TILE
    TOT = NWIN * W_E

    nc = bacc.Bacc("TRN2", target_bir_lowering=False, debug=False,
                   num_devices=NCORES)
    xsrcT = nc.dram_tensor("xsrcT", [D, TOT], BF, kind="ExternalInput")
    eaT = nc.dram_tensor("eaT", [ED, TOT], BF, kind="ExternalInput")
    ohrD = nc.dram_tensor("ohrD", [128, TOT], BF, kind="ExternalInput")
    ohcD = nc.dram_tensor("ohcD", [128, TOT], BF, kind="ExternalInput")
    rowx = nc.dram_tensor("rowx", [ROWS_PER_CORE, D], F32, kind="ExternalInput")
    rowxT = nc.dram_tensor("rowxT", [D, ROWS_PER_CORE], BF, kind="ExternalInput")
    wl = nc.dram_tensor("wl", [D, D], BF, kind="ExternalInput")
    we_ = nc.dram_tensor("we_", [ED, D], BF, kind="ExternalInput")
    wr = nc.dram_tensor("wr", [D, D], BF, kind="ExternalInput")
    attm = nc.dram_tensor("attm", [D, H], BF, kind="ExternalInput")
    biaslr = nc.dram_tensor("biaslr", [128, 1], F32, kind="ExternalInput")
    biasob = nc.dram_tensor("biasob", [128, D], F32, kind="ExternalInput")
    gamb = nc.dram_tensor("gamb", [128, D], F32, kind="ExternalInput")
    betb = nc.dram_tensor("betb", [128, D], F32, kind="ExternalInput")
    out = nc.dram_tensor("out", [ROWS_PER_CORE, D], F32, kind="ExternalOutput")

    with tile.TileContext(nc) as tc:
        with (
            tc.tile_pool(name="const", bufs=1) as cp,
            tc.tile_pool(name="sp3", bufs=3) as sp3,
            tc.tile_pool(name="sp2", bufs=2) as sp2,
            tc.tile_pool(name="work", bufs=3) as wp,
            tc.tile_pool(name="lrp", bufs=14) as lrp,
            tc.tile_pool(name="msgp", bufs=4) as msgp,
            tc.tile_pool(name="epi", bufs=2) as ep,
            tc.tile_pool(name="psB", bufs=2, space="PSUM") as psB,
            tc.tile_pool(name="psS", bufs=2, space="PSUM") as psS,
            tc.tile_pool(name="psD", bufs=3, space="PSUM") as psD,
            tc.tile_pool(name="psF", bufs=1, space="PSUM") as psF,
        ):
            # ---- constants ----
            wl_s = cp.tile([D, D], BF)
            nc.sync.dma_start(wl_s[:], wl[:])
            we_s = cp.tile([ED, D], BF)
            nc.sync.dma_start(we_s[:], we_[:])
            wr_s = cp.tile([D, D], BF)
            nc.sync.dma_start(wr_s[:], wr[:])
            att_s = cp.tile([D, H], BF)
            nc.sync.dma_start(att_s[:], attm[:])
            blr_s = cp.tile([128, 1], F32)
            nc.sync.dma_start(blr_s[:], biaslr[:])
            bob_s = cp.tile([128, D], F32)
            nc.sync.dma_start(bob_s[:], biasob[:])
            gam_s = cp.tile([128, D], F32)
            nc.sync.dma_start(gam_s[:], gamb[:])
            bet_s = cp.tile([128, D], F32)
            nc.sync.dma_start(bet_s[:], betb[:])
            eps_s = cp.tile([128, 1], F32)
            nc.vector.memset(eps_s[:], LN_EPS)
            alpha_s = cp.tile([128, 1], F32)
            nc.vector.memset(alpha_s[:], NEG_SLOPE)

            # ---- prologue: hd table = row_x @ w_r, bf16, [128 rowlocal, w, D]
            rxT_s = cp.tile([D, ROWS_PER_CORE], BF)
            nc.sync.dma_start(rxT_s[:], rowxT[:])
            hd_s = cp.tile([128, NWIN, D], BF)
            for w in range(NWIN):
                hdp = psB.tile([128, TILE], mybir.dt.float32, tag="eB")
                nc.tensor.matmul(hdp[:, 0:D], lhsT=rxT_s[:, w * 128:(w + 1) * 128],
                                 rhs=wr_s[:], start=True, stop=True)
                nc.scalar.copy(hd_s[:, w, :], hdp[:, 0:D])
            xst_s = cp.tile([128, NWIN, D], F32)      # x for LN, staged
            mv_s = cp.tile([128, NWIN, 2], F32)       # per-window mean/var
            on_s = cp.tile([128, NWIN, D], F32)       # normalized out stage

            # ---- main loop over windows (software-pipelined emission:
            # B+score phase of window w is emitted before the C+E/epilogue of
            # window w-1 so the PE has independent work during reduce stalls)
            def emit_bscore(w):
                esl = slice(w * W_E, (w + 1) * W_E)
                xs_s = sp3.tile([D, W_E], BF, tag="xs")
                nc.sync.dma_start(xs_s[:], xsrcT[:, esl])
                ea_s = sp2.tile([ED, W_E], BF, tag="ea")
                nc.sync.dma_start(ea_s[:], eaT[:, esl])
                ohr_s = sp2.tile([128, W_E], BF, tag="ohr")
                nc.sync.dma_start(ohr_s[:], ohrD[:, esl])
                ohc_s = sp3.tile([128, W_E], BF, tag="ohc")
                nc.sync.dma_start(ohc_s[:], ohcD[:, esl])

                SC = psS.tile([128, 4 * NBLK], mybir.dt.float32, tag="sc")
                lrs = []
                for tp in range(0, NT, 2):
                    ts_ = [tp] if tp + 1 >= NT else [tp, tp + 1]
                    Bs = [psB.tile([128, TILE], mybir.dt.float32, tag="eB",
                                   name=f"B_{w}_{tp}_{i}")
                          for i in range(len(ts_))]
                    for wi, (lhsT, rhs_of) in enumerate((
                            (wl_s, xs_s), (we_s, ea_s), (None, ohr_s))):
                        lh = hd_s[:, w, :] if lhsT is None else lhsT[:]
                        for Bt, t in zip(Bs, ts_):
                            csl = slice(t * TILE, (t + 1) * TILE)
                            nc.tensor.matmul(Bt[:], lhsT=lh, rhs=rhs_of[:, csl],
                                             start=(wi == 0), stop=(wi == 2))
                    for Bt, t in zip(Bs, ts_):
                        lr = lrp.tile([128, TILE], BF, tag="lr",
                                      name=f"lr_{w}_{t}")
                        nc.scalar.activation(lr[:], Bt[:], func=AF.Prelu,
                                             bias=blr_s[:], scale=1.0,
                                             alpha=alpha_s[:])
                        lrs.append(lr)
                for t in range(NT):
                    for j in range(4):
                        b = t * 4 + j
                        nc.tensor.matmul(SC[:, b * 4:(b + 1) * 4],
                                         lhsT=lrs[t][:, j * 128:(j + 1) * 128],
                                         rhs=att_s[:], start=True, stop=True)
                ww = wp.tile([128, 4 * NBLK], BF, tag="ww")
                nc.scalar.activation(ww[:], SC[:], func=AF.Exp)
                return xs_s, ohc_s, ww

            def emit_ce(w, xs_s, ohc_s, ww):
                FG = psF.tile([128, 132], mybir.dt.float32, tag="fg")

                def reduce_mms(t, msgt):
                    for j in range(4):
                        b = t * 4 + j
                        bsl = slice(b * 128, (b + 1) * 128)
                        nc.tensor.matmul(FG[:], lhsT=ohc_s[:, bsl],
                                         rhs=msgt[:, j, :],
                                         start=(t == 0 and j == 0),
                                         stop=(t == NT - 1 and j == 3))

                pend_red = []
                for t in range(NT):
                    Dp = psD.tile([128, TILE], mybir.dt.float32, tag="hsD")
                    msgt = msgp.tile([128, 4, 132], BF, tag="msg")
                    for j in range(4):
                        b = t * 4 + j
                        bsl = slice(b * 128, (b + 1) * 128)
                        jsl = slice(j * 128, (j + 1) * 128)
                        nc.tensor.matmul(Dp[:, jsl], lhsT=xs_s[:, bsl],
                                         rhs=wl_s[:], start=True, stop=True)
                    nc.vector.tensor_tensor(
                        out=msgt[:, :, 0:128].rearrange(
                            "p b (h c) -> p b h c", h=H),
                        in0=Dp[:].rearrange("p (b h c) -> p b h c", b=4, h=H),
                        in1=ww[:, t * 16:(t + 1) * 16]
                            .rearrange("p (b h) -> p b h", b=4)[:, :, :, None]
                            .broadcast_to([128, 4, H, C]),
                        op=ALU.mult)
                    nc.vector.tensor_copy(
                        out=msgt[:, :, 128:132],
                        in_=ww[:, t * 16:(t + 1) * 16]
                            .rearrange("p (b h) -> p b h", b=4))
                    pend_red.append((t, msgt))
                    if len(pend_red) > 2:
                        reduce_mms(*pend_red.pop(0))
                for pr in pend_red:
                    reduce_mms(*pr)

                # ---- light window epilogue (DVE only; normalize later)
                rx_s = ep.tile([128, D], F32, tag="rx")
                nc.sync.dma_start(rx_s[:], rowx[w * 128:(w + 1) * 128, :])
                den = ep.tile([128, H], F32, tag="den")
                nc.vector.tensor_scalar(out=den[:], in0=FG[:, 128:132],
                                        scalar1=1e-30, scalar2=None, op0=ALU.max)
                nc.vector.reciprocal(den[:], den[:])
                xln = ep.tile([128, D], F32, tag="xln")
                nc.vector.tensor_tensor(
                    out=xln[:].rearrange("p (h c) -> p h c", h=H),
                    in0=FG[:, 0:128].rearrange("p (h c) -> p h c", h=H),
                    in1=den[:][:, :, None].broadcast_to([128, H, C]),
                    op=ALU.mult)
                nc.vector.tensor_add(xln[:], xln[:], bob_s[:])
                nc.vector.tensor_add(xst_s[:, w, :], xln[:], rx_s[:])
                st = ep.tile([128, 6], F32, tag="st")
                nc.vector.bn_stats(out=st[:], in_=xst_s[:, w, :])
                nc.vector.bn_aggr(out=mv_s[:, w, :], in_=st[:])

            GRP = 5

            def emit_norm_group(g):
                wsl = slice(g * GRP, (g + 1) * GRP)
                rstd_a = ep.tile([128, GRP], F32, tag="rsa")
                nc.scalar.activation(rstd_a[:], mv_s[:, wsl, 1], func=AF.Sqrt,
                                     bias=eps_s[:], scale=1.0)
                nc.vector.reciprocal(rstd_a[:], rstd_a[:])
                for i in range(GRP):
                    w = g * GRP + i
                    nc.vector.tensor_scalar(out=on_s[:, w, :],
                                            in0=xst_s[:, w, :],
                                            scalar1=mv_s[:, w, 0:1],
                                            scalar2=rstd_a[:, i:i + 1],
                                            op0=ALU.subtract, op1=ALU.mult)
                    nc.vector.tensor_mul(on_s[:, w, :], on_s[:, w, :], gam_s[:])
                    nc.vector.tensor_add(on_s[:, w, :], on_s[:, w, :], bet_s[:])
                nc.sync.dma_start(
                    out[g * GRP * 128:(g + 1) * GRP * 128, :]
                    .rearrange("(w p) d -> p w d", p=128),
                    on_s[:, wsl, :])

            pend = []
            done = 0
            for w in range(NWIN):
                pend.append((w, emit_bscore(w)))
                if len(pend) > 2:
                    pw, refs = pend.pop(0)
                    emit_ce(pw, *refs)
                    done = pw + 1
                    if done % GRP == 0:
                        emit_norm_group(done // GRP - 1)
            for pw, refs in pend:
                emit_ce(pw, *refs)
                done = pw + 1
                if done % GRP == 0:
                    emit_norm_group(done // GRP - 1)
    nc.compile()
    return nc


def _host_prep(row_x, token_x, t2r_edge_index, edge_attr_t2r, w_l, b_l, w_r,
               b_r, w_e, att, bias, ln_gamma, ln_beta):
    src = np.ascontiguousarray(t2r_edge_index[0])
    dst = np.ascontiguousarray(t2r_edge_index[1])
    order = np.argsort(dst, kind="stable")
    dst_s = dst[order]
    src_s = src[order]
    win = dst_s // WIN                       # global window id, 0..159
    counts = np.bincount(win, minlength=NW_TOT)
    W_E = int(np.ceil(max(counts.max(), 1) / TILE) * TILE)
    starts = np.zeros(NW_TOT + 1, np.int64)
    np.cumsum(counts, out=starts[1:])
    pos = np.arange(E, dtype=np.int64) - starts[win]
    dst_rel = (dst_s - win * WIN).astype(np.int64)

    TOT = NWIN * W_E

    # constant inputs shared by all cores
    att_mat = np.zeros((D, H), np.float32)
    for h in range(H):
        att_mat[h * C:(h + 1) * C, h] = att[h]
    consts = dict(
        wl=w_l.astype(bf16), we_=w_e.astype(bf16), wr=w_r.astype(bf16),
        attm=att_mat.astype(bf16),
        biaslr=(b_l + b_r).astype(np.float32)[:, None].copy(),
        biasob=np.ascontiguousarray(
            np.broadcast_to((bias + b_l).astype(np.float32), (128, D))),
        gamb=np.ascontiguousarray(
            np.broadcast_to(ln_gamma.astype(np.float32), (128, D))),
        betb=np.ascontiguousarray(
            np.broadcast_to(ln_beta.astype(np.float32), (128, D))),
    )

    rx_pad = np.zeros((ROWS_PAD, D), np.float32)
    rx_pad[:N_ROWS] = row_x

    in_maps = []
    for k in range(NCORES):
        w0 = k * NWIN
        e0, e1 = starts[w0], starts[w0 + NWIN]
        sl = slice(e0, e1)
        slot = (win[sl] - w0) * W_E + pos[sl]          # local padded slot

        xs = np.zeros((TOT, D), bf16)
        xs[slot] = token_x[src_s[sl]].astype(bf16)
        ea = np.zeros((TOT, ED), bf16)
        ea[slot] = edge_attr_t2r[order[sl]].astype(bf16)

        dr = dst_rel[sl]
        ohr = np.zeros((128, TOT), bf16)
        ohr[dr, slot] = bf16(1.0)
        ohc = np.zeros((128, TOT), bf16)
        ohc[slot % 128, (slot // 128) * 128 + dr] = bf16(1.0)

        rx_core = np.ascontiguousarray(
            rx_pad[k * ROWS_PER_CORE:(k + 1) * ROWS_PER_CORE])
        m = dict(
            xsrcT=np.ascontiguousarray(xs.T),
            eaT=np.ascontiguousarray(ea.T),
            ohrD=ohr,
            ohcD=ohc,
            rowx=rx_core,
            rowxT=np.ascontiguousarray(rx_core.T.astype(bf16)),
            **consts,
        )
        in_maps.append(m)
    return W_E, in_maps


def kernel(row_x, token_x, t2r_edge_index, edge_attr_t2r, r2t_edge_index,
           edge_attr_r2t, w_l, b_l, w_r, b_r, w_e, att, bias, ln_gamma,
           ln_beta, **_ignored):
    global last_run
    from concourse.bass_utils import run_bass_kernel_spmd

    row_x = np.asarray(row_x, np.float32)
    token_x = np.asarray(token_x, np.float32)
    t2r_edge_index = np.asarray(t2r_edge_index, np.int32)
    edge_attr_t2r = np.asarray(edge_attr_t2r, np.float32)
    w_l = np.asarray(w_l, np.float32)
    b_l = np.asarray(b_l, np.float32)
    w_r = np.asarray(w_r, np.float32)
    b_r = np.asarray(b_r, np.float32)
    w_e = np.asarray(w_e, np.float32)
    att = np.asarray(att, np.float32)
    bias = np.asarray(bias, np.float32)
    ln_gamma = np.asarray(ln_gamma, np.float32)
    ln_beta = np.asarray(ln_beta, np.float32)

    W_E, in_maps = _host_prep(row_x, token_x, t2r_edge_index, edge_attr_t2r,
                              w_l, b_l, w_r, b_r, w_e, att, bias, ln_gamma,
                              ln_beta)
    if W_E not in _prog_cache:
        _prog_cache[W_E] = _build_program(W_E)
    nc = _prog_cache[W_E]

    last_run = run_bass_kernel_spmd(nc, in_maps, core_ids=list(range(NCORES)))
    outs = [r["out"] for r in last_run.results]
    row_out = np.concatenate(outs, axis=0)[:N_ROWS].astype(np.float32)
    return (row_out, token_x)


# revision 14
# speedup vs baseline: 1.0986x; 1.0986x over previous
"""Trainium2 Bass kernel for nn_GATLayer (GATv2 message passing + LayerNorm).

Strategy (row/edge hybrid parallel, 8 cores):
  - Pad the 20000 destination rows to 20480 = 8 cores x 20 windows x 128 rows.
    Core k owns rows [2560k, 2560(k+1)) and ALL edges pointing at them.
  - Host prep: sort edges by dst, group them into 128-row windows, pad each
    window's edge list to a common W_E (multiple of 512), gather token_x[src]
    and edge_attr per edge, build onehot connectivity matrices (row-major and
    col-major), and ship per-core transposed bf16 streams.
  - Device, per window: e = x_src@w_l + ea@w_e + ohr^T-expand(hd), lrelu
    (Prelu, alpha=0.2), score via block-diag att matmuls into one PSUM bank,
    w = exp(score) once per window (no max-subtraction needed: out =
    sum(hs*w)/sum(w) is softmax-shift-invariant and scores are O(5)),
    msg = hs_col * w, segment-reduce via onehot_col matmuls accumulated in
    PSUM, then divide, residual add, LayerNorm -> 2560 output rows per core.
  - No collectives: each core owns its rows end-to-end; host concatenates.
"""
import numpy as np
import ml_dtypes

bf16 = ml_dtypes.bfloat16

# Problem constants (hardcoded per harness contract)
N_ROWS, N_TOK, E, D, ED, H, C = 20000, 200000, 1_000_000, 128, 64, 4, 32
NEG_SLOPE, LN_EPS = 0.2, 1e-5
NCORES = 8
WIN = 128                       # rows per window (PSUM partition dim)
ROWS_PAD = 20480                # 160 windows
ROWS_PER_CORE = ROWS_PAD // NCORES   # 2560
NWIN = ROWS_PER_CORE // WIN          # 20
NW_TOT = ROWS_PAD // WIN             # 160
TILE = 512                      # edges per PE moving-operand tile

_prog_cache = {}
last_run = None                 # BassKernelResults of the most recent run


def _build_program(W_E):
    import concourse.tile as tile
    from concourse import bacc, mybir

    F32, BF = mybir.dt.float32, mybir.dt.bfloat16
    AF = mybir.ActivationFunctionType
    ALU = mybir.AluOpType

    NBLK = W_E // 128
    NT = W_E // TILE
    TOT = NWIN * W_E

    nc = bacc.Bacc("TRN2", target_bir_lowering=False, debug=False,
                   num_devices=NCORES)
    xsrcT = nc.dram_tensor("xsrcT", [D, TOT], BF, kind="ExternalInput")
    eaT = nc.dram_tensor("eaT", [ED, TOT], BF, kind="ExternalInput")
    ohrD = nc.dram_tensor("ohrD", [128, TOT], BF, kind="ExternalInput")
    ohcD = nc.dram_tensor("ohcD", [128, TOT], BF, kind="ExternalInput")
    rowx = nc.dram_tensor("rowx", [ROWS_PER_CORE, D], F32, kind="ExternalInput")
    rowxT = nc.dram_tensor("rowxT", [D, ROWS_PER_CORE], BF, kind="ExternalInput")
    wl = nc.dram_tensor("wl", [D, D], BF, kind="ExternalInput")
    we_ = nc.dram_tensor("we_", [ED, D], BF, kind="ExternalInput")
    wr = nc.dram_tensor("wr", [D, D], BF, kind="ExternalInput")
    attm = nc.dram_tensor("attm", [D, H], BF, kind="ExternalInput")
    biaslr = nc.dram_tensor("biaslr", [128, 1], F32, kind="ExternalInput")
    biasob = nc.dram_tensor("biasob", [128, D], F32, kind="ExternalInput")
    gamb = nc.dram_tensor("gamb", [128, D], F32, kind="ExternalInput")
    betb = nc.dram_tensor("betb", [128, D], F32, kind="ExternalInput")
    out = nc.dram_tensor("out", [ROWS_PER_CORE, D], F32, kind="ExternalOutput")

    with tile.TileContext(nc) as tc:
        with (
            tc.tile_pool(name="const", bufs=1) as cp,
            tc.tile_pool(name="sp3", bufs=3) as sp3,
            tc.tile_pool(name="sp2", bufs=2) as sp2,
            tc.tile_pool(name="work", bufs=3) as wp,
            tc.tile_pool(name="lrp", bufs=14) as lrp,
            tc.tile_pool(name="epi", bufs=2) as ep,
            tc.tile_pool(name="psB", bufs=2, space="PSUM") as psB,
            tc.tile_pool(name="psS", bufs=2, space="PSUM") as psS,
            tc.tile_pool(name="psD", bufs=3, space="PSUM") as psD,
            tc.tile_pool(name="psF", bufs=1, space="PSUM") as psF,
        ):
            # ---- constants ----
            wl_s = cp.tile([D, D], BF)
            nc.sync.dma_start(wl_s[:], wl[:])
            we_s = cp.tile([ED, D], BF)
            nc.sync.dma_start(we_s[:], we_[:])
            wr_s = cp.tile([D, D], BF)
            nc.sync.dma_start(wr_s[:], wr[:])
            att_s = cp.tile([D, H], BF)
            nc.sync.dma_start(att_s[:], attm[:])
            blr_s = cp.tile([128, 1], F32)
            nc.sync.dma_start(blr_s[:], biaslr[:])
            bob_s = cp.tile([128, D], F32)
            nc.sync.dma_start(bob_s[:], biasob[:])
            gam_s = cp.tile([128, D], F32)
            nc.sync.dma_start(gam_s[:], gamb[:])
            bet_s = cp.tile([128, D], F32)
            nc.sync.dma_start(bet_s[:], betb[:])
            eps_s = cp.tile([128, 1], F32)
            nc.vector.memset(eps_s[:], LN_EPS)
            alpha_s = cp.tile([128, 1], F32)
            nc.vector.memset(alpha_s[:], NEG_SLOPE)

            # ---- prologue: hd table = row_x @ w_r, bf16, [128 rowlocal, w, D]
            rxT_s = cp.tile([D, ROWS_PER_CORE], BF)
            nc.sync.dma_start(rxT_s[:], rowxT[:])
            hd_s = cp.tile([128, NWIN, D], BF)
            for w in range(NWIN):
                hdp = psB.tile([128, TILE], mybir.dt.float32, tag="eB")
                nc.tensor.matmul(hdp[:, 0:D], lhsT=rxT_s[:, w * 128:(w + 1) * 128],
                                 rhs=wr_s[:], start=True, stop=True)
                nc.scalar.copy(hd_s[:, w, :], hdp[:, 0:D])
            xst_s = cp.tile([128, NWIN, D], F32)      # x for LN, staged
            mv_s = cp.tile([128, NWIN, 2], F32)       # per-window mean/var
            on_s = cp.tile([128, NWIN, D], F32)       # normalized out stage

            # ---- main loop over windows (software-pipelined emission:
            # B+score phase of window w is emitted before the C+E/epilogue of
            # window w-1 so the PE has independent work during reduce stalls)
            def emit_bscore(w):
                esl = slice(w * W_E, (w + 1) * W_E)
                xs_s = sp3.tile([D, W_E], BF, tag="xs")
                nc.sync.dma_start(xs_s[:], xsrcT[:, esl])
                ea_s = sp2.tile([ED, W_E], BF, tag="ea")
                nc.scalar.dma_start(ea_s[:], eaT[:, esl])
                ohr_s = sp2.tile([128, W_E], BF, tag="ohr")
                nc.scalar.dma_start(ohr_s[:], ohrD[:, esl])
                ohc_s = sp3.tile([128, W_E], BF, tag="ohc")
                nc.sync.dma_start(ohc_s[:], ohcD[:, esl])

                SC = psS.tile([128, 4 * NBLK], mybir.dt.float32, tag="sc")
                lrs = []
                for tp in range(0, NT, 2):
                    ts_ = [tp] if tp + 1 >= NT else [tp, tp + 1]
                    Bs = [psB.tile([128, TILE], mybir.dt.float32, tag="eB",
                                   name=f"B_{w}_{tp}_{i}")
                          for i in range(len(ts_))]
                    for wi, (lhsT, rhs_of) in enumerate((
                            (wl_s, xs_s), (we_s, ea_s), (None, ohr_s))):
                        lh = hd_s[:, w, :] if lhsT is None else lhsT[:]
                        for Bt, t in zip(Bs, ts_):
                            csl = slice(t * TILE, (t + 1) * TILE)
                            nc.tensor.matmul(Bt[:], lhsT=lh, rhs=rhs_of[:, csl],
                                             start=(wi == 0), stop=(wi == 2))
                    for Bt, t in zip(Bs, ts_):
                        lr = lrp.tile([128, TILE], BF, tag="lr",
                                      name=f"lr_{w}_{t}")
                        nc.scalar.activation(lr[:], Bt[:], func=AF.Prelu,
                                             bias=blr_s[:], scale=1.0,
                                             alpha=alpha_s[:])
                        lrs.append(lr)
                for t in range(NT):
                    for j in range(4):
                        b = t * 4 + j
                        nc.tensor.matmul(SC[:, b * 4:(b + 1) * 4],
                                         lhsT=lrs[t][:, j * 128:(j + 1) * 128],
                                         rhs=att_s[:], start=True, stop=True)
                ww = wp.tile([128, 4 * NBLK], BF, tag="ww")
                nc.scalar.activation(ww[:], SC[:], func=AF.Exp)
                return xs_s, ohc_s, ww

            def emit_ce(w, xs_s, ohc_s, ww):
                FG = psF.tile([128, 132], mybir.dt.float32, tag="fg")

                def reduce_mms(t, msgt):
                    for j in range(4):
                        b = t * 4 + j
                        bsl = slice(b * 128, (b + 1) * 128)
                        nc.tensor.matmul(FG[:], lhsT=ohc_s[:, bsl],
                                         rhs=msgt[:, j, :],
                                         start=(t == 0 and j == 0),
                                         stop=(t == NT - 1 and j == 3))

                prev_red = None
                for t in range(NT):
                    Dp = psD.tile([128, TILE], mybir.dt.float32, tag="hsD")
                    msgt = wp.tile([128, 4, 132], BF, tag="msg")
                    for j in range(4):
                        b = t * 4 + j
                        bsl = slice(b * 128, (b + 1) * 128)
                        jsl = slice(j * 128, (j + 1) * 128)
                        nc.tensor.matmul(Dp[:, jsl], lhsT=xs_s[:, bsl],
                                         rhs=wl_s[:], start=True, stop=True)
                    nc.vector.tensor_tensor(
                        out=msgt[:, :, 0:128].rearrange(
                            "p b (h c) -> p b h c", h=H),
                        in0=Dp[:].rearrange("p (b h c) -> p b h c", b=4, h=H),
                        in1=ww[:, t * 16:(t + 1) * 16]
                            .rearrange("p (b h) -> p b h", b=4)[:, :, :, None]
                            .broadcast_to([128, 4, H, C]),
                        op=ALU.mult)
                    nc.vector.tensor_copy(
                        out=msgt[:, :, 128:132],
                        in_=ww[:, t * 16:(t + 1) * 16]
                            .rearrange("p (b h) -> p b h", b=4))
                    if prev_red is not None:
                        reduce_mms(*prev_red)
                    prev_red = (t, msgt)
                reduce_mms(*prev_red)

                # ---- light window epilogue (DVE only; normalize later)
                rx_s = ep.tile([128, D], F32, tag="rx")
                nc.scalar.dma_start(rx_s[:], rowx[w * 128:(w + 1) * 128, :])
                den = ep.tile([128, H], F32, tag="den")
                nc.vector.tensor_scalar(out=den[:], in0=FG[:, 128:132],
                                        scalar1=1e-30, scalar2=None, op0=ALU.max)
                nc.vector.reciprocal(den[:], den[:])
                xln = ep.tile([128, D], F32, tag="xln")
                nc.vector.tensor_tensor(
                    out=xln[:].rearrange("p (h c) -> p h c", h=H),
                    in0=FG[:, 0:128].rearrange("p (h c) -> p h c", h=H),
                    in1=den[:][:, :, None].broadcast_to([128, H, C]),
                    op=ALU.mult)
                nc.vector.tensor_add(xln[:], xln[:], bob_s[:])
                nc.vector.tensor_add(xst_s[:, w, :], xln[:], rx_s[:])
                st = ep.tile([128, 6], F32, tag="st")
                nc.vector.bn_stats(out=st[:], in_=xst_s[:, w, :])
                nc.vector.bn_aggr(out=mv_s[:, w, :], in_=st[:])

            GRP = 5

            def emit_norm_group(g):
                wsl = slice(g * GRP, (g + 1) * GRP)
                rstd_a = ep.tile([128, GRP], F32, tag="rsa")
                nc.scalar.activation(rstd_a[:], mv_s[:, wsl, 1], func=AF.Sqrt,
                                     bias=eps_s[:], scale=1.0)
                nc.vector.reciprocal(rstd_a[:], rstd_a[:])
                for i in range(GRP):
                    w = g * GRP + i
                    nc.vector.tensor_scalar(out=on_s[:, w, :],
                                            in0=xst_s[:, w, :],
                                            scalar1=mv_s[:, w, 0:1],
                                            scalar2=rstd_a[:, i:i + 1],
                                            op0=ALU.subtract, op1=ALU.mult)
                    nc.vector.tensor_mul(on_s[:, w, :], on_s[:, w, :], gam_s[:])
                    nc.vector.tensor_add(on_s[:, w, :], on_s[:, w, :], bet_s[:])
                nc.sync.dma_start(
                    out[g * GRP * 128:(g + 1) * GRP * 128, :]
                    .rearrange("(w p) d -> p w d", p=128),
                    on_s[:, wsl, :])

            pend = []
            done = 0
            for w in range(NWIN):
                pend.append((w, emit_bscore(w)))
                if len(pend) > 2:
                    pw, refs = pend.pop(0)
                    emit_ce(pw, *refs)
                    done = pw + 1
                    if done % GRP == 0:
                        emit_norm_group(done // GRP - 1)
            for pw, refs in pend:
                emit_ce(pw, *refs)
                done = pw + 1
                if done % GRP == 0:
                    emit_norm_group(done // GRP - 1)
    nc.compile()
    return nc


def _host_prep(row_x, token_x, t2r_edge_index, edge_attr_t2r, w_l, b_l, w_r,
               b_r, w_e, att, bias, ln_gamma, ln_beta):
    src = np.ascontiguousarray(t2r_edge_index[0])
    dst = np.ascontiguousarray(t2r_edge_index[1])
    order = np.argsort(dst, kind="stable")
    dst_s = dst[order]
    src_s = src[order]
    win = dst_s // WIN                       # global window id, 0..159
    counts = np.bincount(win, minlength=NW_TOT)
    W_E = int(np.ceil(max(counts.max(), 1) / TILE) * TILE)
    starts = np.zeros(NW_TOT + 1, np.int64)
    np.cumsum(counts, out=starts[1:])
    pos = np.arange(E, dtype=np.int64) - starts[win]
    dst_rel = (dst_s - win * WIN).astype(np.int64)

    TOT = NWIN * W_E

    # constant inputs shared by all cores
    att_mat = np.zeros((D, H), np.float32)
    for h in range(H):
        att_mat[h * C:(h + 1) * C, h] = att[h]
    consts = dict(
        wl=w_l.astype(bf16), we_=w_e.astype(bf16), wr=w_r.astype(bf16),
        attm=att_mat.astype(bf16),
        biaslr=(b_l + b_r).astype(np.float32)[:, None].copy(),
        biasob=np.ascontiguousarray(
            np.broadcast_to((bias + b_l).astype(np.float32), (128, D))),
        gamb=np.ascontiguousarray(
            np.broadcast_to(ln_gamma.astype(np.float32), (128, D))),
        betb=np.ascontiguousarray(
            np.broadcast_to(ln_beta.astype(np.float32), (128, D))),
    )

    rx_pad = np.zeros((ROWS_PAD, D), np.float32)
    rx_pad[:N_ROWS] = row_x

    in_maps = []
    for k in range(NCORES):
        w0 = k * NWIN
        e0, e1 = starts[w0], starts[w0 + NWIN]
        sl = slice(e0, e1)
        slot = (win[sl] - w0) * W_E + pos[sl]          # local padded slot

        xs = np.zeros((TOT, D), bf16)
        xs[slot] = token_x[src_s[sl]].astype(bf16)
        ea = np.zeros((TOT, ED), bf16)
        ea[slot] = edge_attr_t2r[order[sl]].astype(bf16)

        dr = dst_rel[sl]
        ohr = np.zeros((128, TOT), bf16)
        ohr[dr, slot] = bf16(1.0)
        ohc = np.zeros((128, TOT), bf16)
        ohc[slot % 128, (slot // 128) * 128 + dr] = bf16(1.0)

        rx_core = np.ascontiguousarray(
            rx_pad[k * ROWS_PER_CORE:(k + 1) * ROWS_PER_CORE])
        m = dict(
            xsrcT=np.ascontiguousarray(xs.T),
            eaT=np.ascontiguousarray(ea.T),
            ohrD=ohr,
            ohcD=ohc,
            rowx=rx_core,
            rowxT=np.ascontiguousarray(rx_core.T.astype(bf16)),
            **consts,
        )
        in_maps.append(m)
    return W_E, in_maps


def kernel(row_x, token_x, t2r_edge_index, edge_attr_t2r, r2t_edge_index,
           edge_attr_r2t, w_l, b_l, w_r, b_r, w_e, att, bias, ln_gamma,
           ln_beta, **_ignored):
    global last_run
    from concourse.bass_utils import run_bass_kernel_spmd

    row_x = np.asarray(row_x, np.float32)
    token_x = np.asarray(token_x, np.float32)
    t2r_edge_index = np.asarray(t2r_edge_index, np.int32)
    edge_attr_t2r = np.asarray(edge_attr_t2r, np.float32)
    w_l = np.asarray(w_l, np.float32)
    b_l = np.asarray(b_l, np.float32)
    w_r = np.asarray(w_r, np.float32)
    b_r = np.asarray(b_r, np.float32)
    w_e = np.asarray(w_e, np.float32)
    att = np.asarray(att, np.float32)
    bias = np.asarray(bias, np.float32)
    ln_gamma = np.asarray(ln_gamma, np.float32)
    ln_beta = np.asarray(ln_beta, np.float32)

    W_E, in_maps = _host_prep(row_x, token_x, t2r_edge_index, edge_attr_t2r,
                              w_l, b_l, w_r, b_r, w_e, att, bias, ln_gamma,
                              ln_beta)
    if W_E not in _prog_cache:
        _prog_cache[W_E] = _build_program(W_E)
    nc = _prog_cache[W_E]

    last_run = run_bass_kernel_spmd(nc, in_maps, core_ids=list(range(NCORES)))
    outs = [r["out"] for r in last_run.results]
    row_out = np.concatenate(outs, axis=0)[:N_ROWS].astype(np.float32)
    return (row_out, token_x)
